# revision 13
# baseline (speedup 1.0000x reference)
"""Trainium2 Bass kernel for nn_DecoderLayer_31086973288870.

Full decoder layer (QKV -> causal attention -> out-proj -> LN -> FFN -> LN),
S=2048, D=2048, 16 heads, INNER=8192, batch 1, fp32 reference.

Sharding (8 cores):
  - Attention: tensor-parallel over heads (2 heads/core). QKV column-parallel.
  - Per-head AllToAll (bf16) turns head-shards into seq-shards; it carries the
    UNNORMALIZED PV output plus the softmax denominators, so the renorm is one
    [8,256] reciprocal + 16 broadcast-multiplies after the exchange, and the
    first A2A overlaps the second head's attention compute.
  - Out-proj / LN1 / FFN / LN2: sequence-parallel, full weights streamed from
    HBM in host-prepacked contiguous layouts (>=8KB per-partition lines).
  - Host concatenates the 8 [256, 2048] output slices.

Dtypes: every matmul runs bf16 x bf16 (fp32 PSUM accumulation); residuals,
LayerNorm statistics and softmax sums stay fp32. Simulated end-to-end error
2.6e-3 vs the f64 reference (gate 2e-2).
"""

import math
import sys

import numpy as np

try:
    import concourse.bass as bass  # noqa: F401
except ImportError:  # pragma: no cover - harness containers stage it here
    sys.path.insert(0, "/opt/trn_rl_repo")
    import concourse.bass as bass  # noqa: F401

import ml_dtypes
import concourse.mybir as mybir
import concourse.tile as tile
from concourse import bacc
from concourse.bass_utils import run_bass_kernel_spmd
from concourse.masks import make_identity
from contextlib import ExitStack

S = 2048
D = 2048
HEADS = 16
HD = 128
INNER = 8192
NCORES = 8
HPC = HEADS // NCORES     # heads per core = 2
HDC = HPC * HD            # head dims per core = 256
SC = S // NCORES          # seq rows per core = 256
EPS = 1e-5
RSQ = 1.0 / math.sqrt(float(D))
SKEW = 2                  # attention software pipeline: scores ahead of PV

f32 = mybir.dt.float32
FP = mybir.dt.float32r
bf16 = mybir.dt.bfloat16
AF = mybir.ActivationFunctionType
OP = mybir.AluOpType
AX = mybir.AxisListType

DEBUG = False


def _build(debug=DEBUG, nocc=False):
    nc = bacc.Bacc("TRN2", target_bir_lowering=False, debug=False,
                   num_devices=NCORES)

    def din(name, shape, dt):
        return nc.dram_tensor(name, shape, dt, kind="ExternalInput").ap()

    def dout(name, shape, dt):
        return nc.dram_tensor(name, shape, dt, kind="ExternalOutput").ap()

    xT_d = din("xT", [128, 16, S], bf16)
    xs_d = din("x_slice", [SC, D], f32)
    wq_d = din("wq", [128, 16, HDC], bf16)
    wk_d = din("wk", [128, 16, HDC], bf16)
    wv_d = din("wv", [128, 16, HDC], bf16)
    bq_d = din("bq", [128, HPC], f32)
    bk_d = din("bk", [128, HPC], f32)
    bv_d = din("bv", [1, HDC], bf16)
    linw_d = din("lin_w", [128, 4, 16, 512], bf16)
    linb_d = din("lin_b", [1, D], bf16)
    ff1_d = din("ff1_w", [128, 16, 16, 512], bf16)
    ff1b_d = din("ff1_b", [128, INNER // 128], f32)
    ff2_d = din("ff2_w", [128, 4, 8, 8, 512], bf16)
    ff2b_d = din("ff2_b", [D], f32)
    ln1g_d = din("ln1_g", [D], f32)
    ln1b_d = din("ln1_b", [D], f32)
    ln2g_d = din("ln2_g", [D], f32)
    ln2b_d = din("ln2_b", [D], f32)
    out_d = dout("out_slice", [SC, D], f32)

    with tile.TileContext(nc) as tc, ExitStack() as ctx:
        const = ctx.enter_context(tc.tile_pool(name="const", bufs=1))
        dram = ctx.enter_context(tc.tile_pool(name="dram", bufs=1, space="DRAM"))
        stat = ctx.enter_context(tc.tile_pool(name="stat", bufs=6))
        gbp = ctx.enter_context(tc.tile_pool(name="gb", bufs=2))

        ident_f = const.tile([128, 128], f32)
        make_identity(nc, ident_f[:])
        onesf = const.tile([128, 128], f32)
        nc.gpsimd.memset(onesf[:], 1.0)
        ones_col = const.tile([128, 1], bf16)
        nc.vector.tensor_copy(ones_col[:], onesf[:, 0:1])
        ones_row = const.tile([1, 128], bf16)
        nc.vector.tensor_copy(ones_row[:], onesf[0:1, :])
        eps_sb = const.tile([128, 1], f32)
        nc.gpsimd.memset(eps_sb[:], EPS)
        # maskbig[i, u] = 1.0 iff u >= i + 384 else 0; slice [384-d : 896-d]
        # is the multiplicative "keep j >= i + delta" causal mask.
        maskf = const.tile([128, 896], f32)
        nc.gpsimd.memset(maskf[:], 1.0)
        nc.gpsimd.affine_select(
            out=maskf[:], in_=maskf[:], compare_op=OP.is_ge, fill=0.0,
            base=-384, channel_multiplier=-1, pattern=[[1, 896]])
        maskbig = const.tile([128, 896], bf16)
        nc.vector.tensor_copy(maskbig[:], maskf[:])

        bq_sb = const.tile([128, HPC], f32)
        nc.sync.dma_start(bq_sb[:], bq_d)
        bk_sb = const.tile([128, HPC], f32)
        nc.sync.dma_start(bk_sb[:], bk_d)
        bv_sb = const.tile([1, HDC], bf16)
        nc.sync.dma_start(bv_sb[:], bv_d)
        ff1b_sb = const.tile([128, INNER // 128], f32)
        nc.sync.dma_start(ff1b_sb[:], ff1b_d)
        linb_sb = const.tile([1, D], bf16)
        nc.sync.dma_start(linb_sb[:], linb_d)

        def bcast_tile(pool, row_d, tag, bufs=1):
            """[D] param from DRAM -> [128, D] bf16 broadcast tile."""
            t = pool.tile([128, D], bf16, tag=tag, name=tag, bufs=bufs)
            nc.gpsimd.dma_start(t[0:1, :], row_d[None, :])
            nc.gpsimd.partition_broadcast(t[:], t[0:1, :])
            return t

        B2f = bcast_tile(const, ff2b_d, "B2f")
        G1 = bcast_tile(gbp, ln1g_d, "G")
        B1 = bcast_tile(gbp, ln1b_d, "B")

        def layernorm(u_tiles, G, B, scope, out_tiles, sbufs=2):
            for ut, o in zip(u_tiles, out_tiles):
                musum = stat.tile([128, 1], f32, tag="musum", name="musum")
                nc.vector.reduce_sum(musum[:], ut[:], axis=AX.X)
                sqsum = stat.tile([128, 1], f32, tag="sqsum", name="sqsum")
                scratch = scope.tile([128, D], f32, tag="ln_scratch",
                                     name="ln_scratch", bufs=sbufs)
                nc.scalar.activation(scratch[:], ut[:], AF.Square,
                                     accum_out=sqsum[:])
                mu = stat.tile([128, 1], f32, tag="mu", name="mu")
                nc.vector.tensor_scalar(mu[:], musum[:], 1.0 / D, None, OP.mult)
                ex2 = stat.tile([128, 1], f32, tag="ex2", name="ex2")
                nc.vector.tensor_scalar(ex2[:], sqsum[:], 1.0 / D, None,
                                        OP.mult)
                mu2 = stat.tile([128, 1], f32, tag="mu2", name="mu2")
                nc.vector.tensor_tensor(mu2[:], mu[:], mu[:], OP.mult)
                var = stat.tile([128, 1], f32, tag="var", name="var")
                nc.vector.tensor_tensor(var[:], ex2[:], mu2[:], OP.subtract)
                std = stat.tile([128, 1], f32, tag="std", name="std")
                nc.scalar.activation(std[:], var[:], AF.Sqrt, bias=eps_sb[:])
                rstd = stat.tile([128, 1], f32, tag="rstd", name="rstd")
                nc.vector.reciprocal(rstd[:], std[:])
                nc.vector.tensor_scalar(o[:], ut[:], mu[:], rstd[:],
                                        OP.subtract, OP.mult)
                nc.vector.tensor_tensor(o[:], o[:], G[:], OP.mult)
                nc.vector.tensor_tensor(o[:], o[:], B[:], OP.add)

        # A2A buffers: one exchange per local head; row 128 carries the
        # softmax denominators for that head.
        a2a_in = [dram.tile([NCORES, 130, SC], bf16, tag=f"a2ain{h}",
                            name=f"a2ain{h}") for h in range(HPC)]
        a2a_out = [dram.tile([NCORES, 130, SC], bf16, tag=f"a2aout{h}",
                             name=f"a2aout{h}") for h in range(HPC)]

        # ======== pools that live through QKV + attention ========
        qkv_keep = ExitStack()
        v_pool = qkv_keep.enter_context(tc.tile_pool(name="v", bufs=1))
        qk_pool = qkv_keep.enter_context(tc.tile_pool(name="qk", bufs=1))
        ot_pool = qkv_keep.enter_context(tc.tile_pool(name="oT", bufs=1))
        ptp = qkv_keep.enter_context(tc.tile_pool(name="pT", bufs=5))
        att_ps = qkv_keep.enter_context(
            tc.tile_pool(name="att_ps", bufs=1, space="PSUM"))

        v_sb = []
        qT = [None] * HPC
        kT = [None] * HPC
        oT = [ot_pool.tile([128, S], bf16, tag=f"oT{h}", name=f"oT{h}")
              for h in range(HPC)]
        seT = [ot_pool.tile([1, S], bf16, tag=f"seT{h}", name=f"seT{h}")
               for h in range(HPC)]

        xw_keep = ExitStack()
        xwp = xw_keep.enter_context(tc.tile_pool(name="xw", bufs=1))
        pp = xw_keep.enter_context(
            tc.tile_pool(name="qkv_ps", bufs=1, space="PSUM"))

        xT_sb = []
        for ch in range(2):
            t = xwp.tile([128, 8, S], bf16, tag=f"xT{ch}", name=f"xT{ch}")
            nc.sync.dma_start(t[:], xT_d[:, ch * 8:(ch + 1) * 8, :])
            xT_sb.append(t)

        def xts(kt):
            return xT_sb[kt // 8][:, kt % 8, :]

        wq_sb = xwp.tile([128, 16, HDC], bf16, tag="wq")
        nc.sync.dma_start(wq_sb[:], wq_d)
        wk_sb = xwp.tile([128, 16, HDC], bf16, tag="wk")
        nc.sync.dma_start(wk_sb[:], wk_d)
        wv_sb = xwp.tile([128, 16, HDC], bf16, tag="wv")
        nc.sync.dma_start(wv_sb[:], wv_d)

        def qkv_head(h):
            for (w_sb, b_sb, dst_list, nmm) in (
                    (wk_sb, bk_sb, kT, "k"), (wq_sb, bq_sb, qT, "q")):
                dst = qk_pool.tile([128, S], bf16, tag=f"{nmm}T{h}",
                                   name=f"{nmm}T{h}")
                for qs in range(4):
                    ps = pp.tile([128, 512], f32, tag="qk_ps",
                                 name="qk_ps", bufs=2)
                    for kt in range(16):
                        nc.tensor.matmul(
                            ps[:], w_sb[:, kt, h * 128:(h + 1) * 128],
                            xts(kt)[:, qs * 512:(qs + 1) * 512],
                            start=(kt == 0), stop=(kt == 15))
                    nc.vector.tensor_scalar(
                        dst[:, qs * 512:(qs + 1) * 512], ps[:],
                        b_sb[:, h:h + 1], None, OP.add)
                dst_list[h] = dst

        def attention_head(h):
            # software-pipelined: scores run SKEW tiles ahead of PV/sum
            for qs in range(4):
                kmax = 4 * qs + 4
                o_ps = att_ps.tile([128, 512], f32, tag="o_ps",
                                   name="o_ps", bufs=2)
                se_ps = att_ps.tile([1, 512], f32, tag="se_ps",
                                    name="se_ps", bufs=1)
                pts = {}
                for i in range(kmax + SKEW):
                    if i < kmax:
                        kt = i
                        s_ps = att_ps.tile([128, 512], f32, tag="s_ps",
                                           name="s_ps", bufs=3)
                        nc.tensor.matmul(
                            s_ps[:], kT[h][:, kt * 128:(kt + 1) * 128],
                            qT[h][:, qs * 512:(qs + 1) * 512],
                            start=True, stop=True)
                        pt = ptp.tile([128, 512], bf16, tag="pt", name="pt")
                        nc.scalar.activation(pt[:], s_ps[:], AF.Exp,
                                             scale=RSQ)
                        delta = kt * 128 - qs * 512
                        if delta >= 0:
                            nc.vector.tensor_tensor(
                                pt[:], pt[:],
                                maskbig[:, 384 - delta:896 - delta], OP.mult)
                        pts[kt] = pt
                    if i >= SKEW:
                        kt = i - SKEW
                        nc.tensor.matmul(
                            o_ps[:], v_sb[kt][:, h * 128:(h + 1) * 128],
                            pts[kt][:], start=(kt == 0),
                            stop=(kt == kmax - 1))
                        nc.tensor.matmul(
                            se_ps[:], ones_col[:], pts[kt][:],
                            start=(kt == 0), stop=(kt == kmax - 1))
                        del pts[kt]
                nc.vector.tensor_copy(
                    oT[h][:, qs * 512:(qs + 1) * 512], o_ps[:])
                nc.vector.tensor_copy(
                    seT[h][:, qs * 512:(qs + 1) * 512], se_ps[:])

        def launch_a2a(h):
            nc.sync.dma_start(
                a2a_in[h][:, 0:128, :].rearrange("c r s -> r c s"),
                oT[h][:].rearrange("r (c s) -> r c s", c=NCORES))
            nc.sync.dma_start(
                a2a_in[h][:, 128:129, :].rearrange("c r s -> r c s"),
                seT[h][:].rearrange("r (c s) -> r c s", c=NCORES))
            if nocc:
                nc.sync.dma_start(a2a_out[h][:], a2a_in[h][:])
            else:
                nc.gpsimd.collective_compute(
                    "AllToAll", OP.bypass,
                    replica_groups=[list(range(NCORES))],
                    ins=[a2a_in[h][:]], outs=[a2a_out[h][:]])

        # ---------------- Phase 1: V projection ------------------------
        for st in range(16):
            ps = pp.tile([128, 512], f32, tag="qk_ps", name="qk_ps", bufs=2)
            for kt in range(16):
                nc.tensor.matmul(
                    ps[:, 0:HDC], xts(kt)[:, st * 128:(st + 1) * 128],
                    wv_sb[:, kt, :], start=(kt == 0), stop=False)
            nc.tensor.matmul(ps[:, 0:HDC], ones_row[:], bv_sb[:],
                             start=False, stop=True)
            vt = v_pool.tile([128, HDC], bf16, tag=f"v{st}", name=f"v{st}")
            nc.vector.tensor_copy(vt[:], ps[:, 0:HDC])
            v_sb.append(vt)

        # ---------------- Phase 2: per-head QKV + attention + A2A ------
        qkv_head(0)
        attention_head(0)
        launch_a2a(0)
        qkv_head(1)
        xw_keep.close()       # x / QKV weights dead; frees 11MB + psum
        attention_head(1)
        launch_a2a(1)
        qkv_keep.close()      # q/k/v/pt/oT staged; attention pools die

        # ---------------- Phase 3: renorm + output projection ----------
        resA = ExitStack()
        res_pool = resA.enter_context(tc.tile_pool(name="res", bufs=1))
        h1b = [res_pool.tile([128, D], f32, tag=f"h1b{m}", name=f"h1b{m}")
               for m in range(2)]
        u2 = [res_pool.tile([128, D], f32, tag=f"u2{m}", name=f"u2{m}")
              for m in range(2)]
        h1T_keep = ExitStack()
        h1Tp = h1T_keep.enter_context(tc.tile_pool(name="h1T", bufs=1))
        h1T = [h1Tp.tile([128, SC], bf16, tag=f"h1T{kt}", name=f"h1T{kt}")
               for kt in range(16)]
        lw_keep = ExitStack()
        lwp = lw_keep.enter_context(tc.tile_pool(name="linw", bufs=1))
        lw_sb = []
        for n in range(4):
            t = lwp.tile([128, 16, 512], bf16, tag=f"lw{n}", name=f"lw{n}")
            nc.gpsimd.dma_start(t[:], linw_d[:, n])
            lw_sb.append(t)
        up_keep = ExitStack()
        up = up_keep.enter_context(tc.tile_pool(name="up", bufs=1))
        u_tiles = [up.tile([128, D], f32, tag=f"u{m}", name=f"u{m}")
                   for m in range(2)]
        xs_sb = []
        for m in range(2):
            t = up.tile([128, D], f32, tag=f"xs{m}", name=f"xs{m}")
            nc.sync.dma_start(t[:], xs_d[m * 128:(m + 1) * 128, :])
            xs_sb.append(t)

        glist = [2 * cs + h for h in range(HPC) for cs in range(NCORES)]
        with tc.tile_pool(name="ofT", bufs=1) as ofp, \
             tc.tile_pool(name="bcp", bufs=4) as bcp, \
             tc.tile_pool(name="op_ps", bufs=4, space="PSUM") as opp, \
             tc.tile_pool(name="tr_ps", bufs=4, space="PSUM") as tpp:
            ofT = []
            rec8 = []
            for h in range(HPC):
                t = ofp.tile([128, S], bf16, tag=f"ofT{h}", name=f"ofT{h}")
                nc.sync.dma_start(
                    t[:].rearrange("r (c s) -> r c s", c=NCORES),
                    a2a_out[h][:, 0:128, :].rearrange("c r s -> r c s"))
                ofT.append(t)
                se8 = ofp.tile([NCORES, SC], bf16, tag=f"se8{h}",
                               name=f"se8{h}")
                nc.sync.dma_start(
                    se8[:],
                    a2a_out[h][:, 128:129, :].rearrange("c r s -> (c r) s"))
                r8 = ofp.tile([NCORES, SC], f32, tag=f"rec8{h}",
                              name=f"rec8{h}")
                nc.vector.reciprocal(r8[:], se8[:])
                # flatten to one partition so partition_broadcast can source it
                rrow = ofp.tile([1, S], f32, tag=f"rrow{h}", name=f"rrow{h}")
                nc.sync.dma_start(
                    rrow[:].rearrange("r (c s) -> r c s", c=NCORES), r8[:])
                rec8.append(rrow)
            # normalize: oT[g] *= 1/se[g]  (global head g = 2*csrc + h)
            for g in glist:
                h, cs = g % 2, g // 2
                bc = bcp.tile([128, SC], f32, tag="bc", name="bc")
                nc.gpsimd.partition_broadcast(
                    bc[:], rec8[h][0:1, cs * SC:(cs + 1) * SC])
                nc.vector.tensor_tensor(
                    ofT[h][:, cs * SC:(cs + 1) * SC],
                    ofT[h][:, cs * SC:(cs + 1) * SC], bc[:], OP.mult)

            for m in range(2):
                for n in range(4):
                    ps = opp.tile([128, 512], f32, tag="op_ps", name="op_ps")
                    for gi, g in enumerate(glist):
                        h, cs = g % 2, g // 2
                        nc.tensor.matmul(
                            ps[:],
                            ofT[h][:,
                                   cs * SC + m * 128:cs * SC + (m + 1) * 128],
                            lw_sb[n][:, g, :],
                            start=(gi == 0), stop=False)
                    nc.tensor.matmul(ps[:], ones_row[:],
                                     linb_sb[:, n * 512:(n + 1) * 512],
                                     start=False, stop=True)
                    nc.vector.tensor_tensor(
                        u_tiles[m][:, n * 512:(n + 1) * 512], ps[:],
                        xs_sb[m][:, n * 512:(n + 1) * 512], OP.add)
                # LN1 (in place: u becomes h1) + transposes for half m
                # overlap the other half's out-proj matmuls
                layernorm([u_tiles[m]], G1, B1, up, [u_tiles[m]], sbufs=1)
                nc.vector.tensor_tensor(h1b[m][:], u_tiles[m][:], B2f[:],
                                        OP.add)
                for kt in range(16):
                    tp = tpp.tile([128, 128], f32, tag="tr_ps", name="tr_ps")
                    nc.tensor.transpose(
                        tp[:], u_tiles[m][:, kt * 128:(kt + 1) * 128],
                        ident_f[:])
                    nc.vector.tensor_copy(
                        h1T[kt][:, m * 128:(m + 1) * 128], tp[:])
        up_keep.close()
        lw_keep.close()

        # ---------------- Phase 4: FFN1 (sequence-parallel) ------------
        gi_keep = ExitStack()
        gip = gi_keep.enter_context(tc.tile_pool(name="gi", bufs=1))
        ginner = []
        with tc.tile_pool(name="w1", bufs=2) as w1p, \
             tc.tile_pool(name="f1_ps", bufs=8, space="PSUM") as fpp:
            for ib in range(16):
                w1t = w1p.tile([128, 16, 512], bf16, tag="w1")
                nc.gpsimd.dma_start(w1t[:], ff1_d[:, ib])
                for ms in range(4):
                    it = ib * 4 + ms
                    ps = fpp.tile([128, SC], f32, tag="f1_ps", name="f1_ps")
                    for kt in range(16):
                        nc.tensor.matmul(
                            ps[:], w1t[:, kt, ms * 128:(ms + 1) * 128],
                            h1T[kt][:], start=(kt == 0), stop=(kt == 15))
                    g = gip.tile([128, SC], bf16, tag=f"gi{it}",
                                 name=f"gi{it}")
                    nc.scalar.activation(g[:], ps[:], AF.Gelu,
                                         bias=ff1b_sb[:, it:it + 1])
                    ginner.append(g)

        # ---------------- Phase 5: FFN2 + LN2 + store ------------------
        with tc.tile_pool(name="w2", bufs=3) as w2p, \
             tc.tile_pool(name="f2_ps", bufs=4, space="PSUM") as f2p:
            for n in range(4):
                pss = [f2p.tile([128, 512], f32, tag=f"f2_ps{m}",
                                name=f"f2ps{m}") for m in range(2)]
                for kc in range(8):
                    w2t = w2p.tile([128, 8, 512], bf16, tag="w2")
                    nc.gpsimd.dma_start(w2t[:], ff2_d[:, n, kc])
                    for m in range(2):
                        for k2 in range(8):
                            kt = kc * 8 + k2
                            nc.tensor.matmul(
                                pss[m][:],
                                ginner[kt][:, m * 128:(m + 1) * 128],
                                w2t[:, k2, :],
                                start=(kt == 0), stop=(kt == 63))
                for m in range(2):
                    nc.vector.tensor_tensor(
                        u2[m][:, n * 512:(n + 1) * 512], pss[m][:],
                        h1b[m][:, n * 512:(n + 1) * 512], OP.add)

            G2 = bcast_tile(gbp, ln2g_d, "G")
            B2 = bcast_tile(gbp, ln2b_d, "B")
            layernorm(u2, G2, B2, w2p, u2)
            for m in range(2):
                nc.sync.dma_start(out_d[m * 128:(m + 1) * 128, :], u2[m][:])
        gi_keep.close()
        h1T_keep.close()
        resA.close()

    nc.compile()
    return nc


_NC_CACHE = {}


def _get_nc(debug=DEBUG, nocc=False, **kw):
    key = (debug, nocc, tuple(sorted(kw.items())))
    if key not in _NC_CACHE:
        _NC_CACHE[key] = _build(debug, nocc, **kw)
    return _NC_CACHE[key]


def _pack_kc(w, kt, rest):
    """[K, ...rest] -> [128, kt, ...rest] with K = kt*128, p fastest on K."""
    return np.ascontiguousarray(
        w.reshape(kt, 128, *rest).transpose(1, 0, *range(2, 2 + len(rest))))


def make_in_maps(x, C_w, C_b, lin_w, lin_b, ff1_w, ff1_b, ff2_w, ff2_b,
                 ln1_g, ln1_b, ln2_g, ln2_b):
    def bfc(a):
        return np.ascontiguousarray(np.asarray(a)).astype(ml_dtypes.bfloat16)

    x2 = np.asarray(x, dtype=np.float32)[0]            # [S, D]
    xT = bfc(x2.T).reshape(16, 128, S).transpose(1, 0, 2)   # [128,16,S]
    C_w = np.asarray(C_w, dtype=np.float32)
    C_b = np.asarray(C_b, dtype=np.float32)
    lin_pack = bfc(lin_w).reshape(16, 128, 4, 512).transpose(1, 2, 0, 3)
    ff1_pack = bfc(ff1_w).reshape(16, 128, 16, 512).transpose(1, 2, 0, 3)
    ff2_pack = bfc(ff2_w).reshape(8, 8, 128, 4, 512).transpose(2, 3, 0, 1, 4)
    common = {
        "xT": np.ascontiguousarray(xT),
        "lin_w": np.ascontiguousarray(lin_pack),
        "lin_b": bfc(lin_b)[None, :],
        "ff1_w": np.ascontiguousarray(ff1_pack),
        "ff1_b": np.ascontiguousarray(
            np.asarray(ff1_b, np.float32).reshape(64, 128).T),
        "ff2_w": np.ascontiguousarray(ff2_pack),
        "ff2_b": np.asarray(ff2_b, dtype=np.float32),
        "ln1_g": np.asarray(ln1_g, dtype=np.float32),
        "ln1_b": np.asarray(ln1_b, dtype=np.float32),
        "ln2_g": np.asarray(ln2_g, dtype=np.float32),
        "ln2_b": np.asarray(ln2_b, dtype=np.float32),
    }
    in_maps = []
    for c in range(NCORES):
        sl = slice(c * HDC, (c + 1) * HDC)
        m = dict(common)
        m["wq"] = _pack_kc(bfc(C_w[:, sl]), 16, [HDC])
        m["wk"] = _pack_kc(bfc(C_w[:, D:][:, sl]), 16, [HDC])
        m["wv"] = _pack_kc(bfc(C_w[:, 2 * D:][:, sl]), 16, [HDC])
        m["bq"] = np.ascontiguousarray(C_b[sl].reshape(HPC, 128).T)
        m["bk"] = np.ascontiguousarray(C_b[D:][sl].reshape(HPC, 128).T)
        m["bv"] = bfc(C_b[2 * D:][sl])[None, :]
        m["x_slice"] = np.ascontiguousarray(x2[c * SC:(c + 1) * SC, :])
        in_maps.append(m)
    return in_maps


def run(in_maps, debug=DEBUG):
    nc = _get_nc(debug)
    return run_bass_kernel_spmd(nc, in_maps, list(range(NCORES)))


def kernel(**inputs):
    in_maps = make_in_maps(**inputs)
    res = run(in_maps)
    out = np.concatenate(
        [res.results[c]["out_slice"] for c in range(NCORES)], axis=0)
    return out.reshape(1, S, D).astype(np.float32)
